# revision 1
# baseline (speedup 1.0000x reference)
"""BFP (block floating point) activation quantization kernel for Trainium2.

Problem: NCHW input [32, 256, 56, 56] f32. Blocks of 8 consecutive channels
share one exponent (at each (n, h, w) position). Per block:
    maxabs = max |x_i|
    p      = 2^floor(log2(maxabs))        (exponent-only part of maxabs)
    s      = p / 4                        (scale; mantissa_bits = 3)
    q_i    = clip(round_half_even(x_i/s), -7, 7) * s   (0 for all-zero blocks)

Strategy (per core; batch dim sharded 4 per core across 8 cores):
  Layout: partition p = (n, cb) [n and cb are adjacent in DRAM so they merge
  into one DMA dim]; free dims = (ch in [0,8), sp chunk of spatial).

  Math (all exact in fp32, bit-identical to the reference):
    pb   = bits(maxabs) & 0xFF800000          -> p (power of two)
    invp = bits^-1(0x7F000000 - pb)           -> 1/p (exact)
    r    = x * invp                           (exact: |r| < 2)
    t    = (r + 1.5*2^21) - 1.5*2^21          -> round-half-even to grid 1/4
    w    = clip(t, -1.75, 1.75)               -> clip(round(x/s),-7,7) / 4
    q    = w * p                              (exact)
  Zero blocks: pb = 0 so q = w * 0 = 0.

  Engine placement is tuned against HW-measured rates (per core, per run):
  DVE ~1.67x drain factor on streaming; Pool broadcast-TT is ~3.5x slower
  than DVE; ACT dense Copy is cheap (~25us/pass) but broadcast-input ACT is
  unusable. DMA measured ~225GB/s at 784B runs. The two big multiplies are
  column-split between DVE and Pool to balance engine totals.
"""

import numpy as np

N, C, H, W = 32, 256, 56, 56
NCORES = 8
NPC = N // NCORES        # batches per core
S = H * W                # 3136
BLK = 8
CB = C // BLK            # 32 channel blocks; partition = (n, cb) -> 4*32 = 128
LT = 196                 # DMA tile spatial extent (descriptor run = 4*LT bytes)
LC = 196                 # compute chunk spatial extent (must divide LT)
BIG_BUFS = 12            # X-tile pipeline depth (in units of LT tiles)
MUL_POOL_FRAC = 0.0      # fraction of r=x*invp columns done on Pool
PMUL_POOL_FRAC = 0.7     # fraction of q=w*p columns done on Pool
C2 = 3145728.0           # 1.5 * 2^21: round-to-nearest-grid-1/4 magic constant

_cached = {}


def _splits(frac):
    """Column split of [0, LC): DVE gets [cut, LC), Pool gets [0, cut)."""
    cut = int(round(frac * LC / 4)) * 4
    return cut


def _build(bench_reps=None):
    import concourse.bacc as bacc
    import concourse.tile as tile
    import concourse.mybir as mybir

    assert S % LT == 0 and LT % LC == 0
    NT = S // LT             # number of DMA tiles
    CPT = LT // LC           # compute chunks per tile
    NCH = NT * CPT           # total compute chunks

    nc = bacc.Bacc("TRN2", target_bir_lowering=False, debug=False)
    x_d = nc.dram_tensor("x", [NPC, C, S], mybir.dt.float32, kind="ExternalInput").ap()
    q_d = nc.dram_tensor("q", [NPC, C, S], mybir.dt.float32, kind="ExternalOutput").ap()
    xv = x_d.rearrange("n (cb ch) s -> (n cb) ch s", ch=BLK)
    qv = q_d.rearrange("n (cb ch) s -> (n cb) ch s", ch=BLK)

    f32, i32 = mybir.dt.float32, mybir.dt.int32
    Alu, Act = mybir.AluOpType, mybir.ActivationFunctionType

    mul_cut = _splits(MUL_POOL_FRAC)
    pmul_cut = _splits(PMUL_POOL_FRAC)

    with tile.TileContext(nc) as tc:
        with (
            tc.tile_pool(name="big", bufs=BIG_BUFS) as big,
            tc.tile_pool(name="small", bufs=BIG_BUFS * CPT) as small,
            tc.tile_pool(name="consts", bufs=1) as consts,
        ):
            c7f = consts.tile([128, 1], i32)
            nc.vector.memset(c7f[:], 0x7F000000)

            Xs, ms, pbs, invps = {}, {}, {}, {}

            def xslice(g):
                # chunk g lives in tile T at sub-range [j*LC, (j+1)*LC)
                T, j = divmod(g, CPT)
                return Xs[T][:, :, j * LC:(j + 1) * LC]

            def st_dma_in(g):
                T, j = divmod(g, CPT)
                if j == 0:
                    Xs[T] = big.tile([128, BLK, LT], f32, tag="X", name=f"X{T}")
                    nc.sync.dma_start(Xs[T][:], xv[:, :, T * LT:(T + 1) * LT])

            def st_reduce(g):
                ms[g] = small.tile([128, LC], f32, tag="m", name=f"m{g}")
                nc.vector.tensor_reduce(
                    out=ms[g][:], in_=xslice(g).rearrange("p ch sp -> p sp ch"),
                    axis=mybir.AxisListType.X, op=Alu.max,
                    apply_absolute_value=True,
                )

            def st_params(g):
                # int32 bitwise only exists on DVE; int32 subtract ok on Pool
                pbs[g] = small.tile([128, LC], i32, tag="pb", name=f"pb{g}")
                nc.vector.tensor_scalar(
                    out=pbs[g][:], in0=ms[g][:].bitcast(i32),
                    scalar1=-8388608,  # 0xFF800000 as int32
                    scalar2=None, op0=Alu.bitwise_and,
                )
                invps[g] = small.tile([128, LC], i32, tag="invp", name=f"invp{g}")
                nc.gpsimd.tensor_tensor(
                    out=invps[g][:], in0=c7f[:].broadcast_to([128, LC]),
                    in1=pbs[g][:], op=Alu.subtract,
                )

            def _split_tt(g, other, cut):
                """in-place X = X * other_bcast, columns [0,cut) on Pool and
                [cut, LC) on DVE."""
                Xg = xslice(g)
                ob = other[:].bitcast(f32).unsqueeze(1)
                if cut > 0:
                    nc.gpsimd.tensor_tensor(
                        out=Xg[:, :, 0:cut], in0=Xg[:, :, 0:cut],
                        in1=ob[:, :, 0:cut].broadcast_to([128, BLK, cut]),
                        op=Alu.mult,
                    )
                if cut < LC:
                    nc.vector.tensor_tensor(
                        out=Xg[:, :, cut:LC], in0=Xg[:, :, cut:LC],
                        in1=ob[:, :, cut:LC].broadcast_to([128, BLK, LC - cut]),
                        op=Alu.mult,
                    )

            def st_mul(g):
                _split_tt(g, invps[g], mul_cut)     # r = x / p (exact)

            def st_act1(g):
                # t = r + C2  (round-half-even to grid 1/4)
                nc.scalar.activation(out=xslice(g), in_=xslice(g), func=Act.Copy, bias=C2, scale=1.0)

            def st_act2(g):
                nc.scalar.activation(out=xslice(g), in_=xslice(g), func=Act.Copy, bias=-C2, scale=1.0)

            def st_clip(g):
                nc.vector.tensor_scalar(
                    out=xslice(g), in0=xslice(g), scalar1=-1.75, scalar2=1.75,
                    op0=Alu.max, op1=Alu.min,
                )

            def st_pmul(g):
                _split_tt(g, pbs[g], pmul_cut)      # q = w * p (exact)

            def st_dma_out(g):
                T, j = divmod(g, CPT)
                if j == CPT - 1:
                    nc.sync.dma_start(qv[:, :, T * LT:(T + 1) * LT], Xs[T][:])
                gg = g  # free small tiles for this chunk
                del ms[gg], pbs[gg], invps[gg]

            stages = [st_dma_in, st_reduce, st_params, st_mul,
                      st_act1, st_act2, st_clip, st_pmul, st_dma_out]

            def ladder():
                # software-pipelined emission so every engine's stream
                # interleaves chunks; an unmet wait never blocks younger
                # ready work.
                for t in range(NCH + len(stages) - 1):
                    for si, stage in enumerate(stages):
                        g = t - si
                        if 0 <= g < NCH:
                            stage(g)

            if bench_reps:
                with tc.For_i(0, bench_reps, 1):
                    ladder()
            else:
                ladder()
    nc.compile()
    return nc


def get_nc():
    if "nc" not in _cached:
        _cached["nc"] = _build()
    return _cached["nc"]


def kernel(activations, _trace=False):
    from concourse.bass_utils import run_bass_kernel_spmd

    nc = get_nc()
    a = np.ascontiguousarray(activations, dtype=np.float32).reshape(N, C, S)
    in_maps = [{"x": a[i * NPC:(i + 1) * NPC]} for i in range(NCORES)]
    res = run_bass_kernel_spmd(nc, in_maps, core_ids=list(range(NCORES)), trace=_trace)
    out = np.concatenate([r["q"] for r in res.results], axis=0)
    if _trace:
        kernel.last_results = res
    return out.reshape(N, C, H, W)

